# revision 3
# baseline (speedup 1.0000x reference)
"""MoE feed-forward (top-2 routing + shared expert) on 8 Trainium2 cores.

Strategy (expert parallel):
  - Host computes the router (tiny [T,D]@[D,E] matmul), top-2 expert ids and
    renormalized gates, then dispatches each expert's tokens (transposed,
    capacity-padded) to the core that owns that expert's weights.
  - Core e computes  ye = (silu(xe@w1_e) * (xe@w3_e)) @ w2_e, row-scaled by the
    gate, plus a 1/8 token-slice of the always-active shared expert.
  - Host scatter-adds routed outputs into the shared-expert output.

All matmuls run in bf16 (fp32 PSUM accumulation). bf16 keeps the PE at one
moving column per cycle like fp32r, but its LDWEIGHTS goes through the fast
weight load path (~53ns vs ~191ns), so narrow token chunks no longer pay a
weight-load floor, and every DMA byte count halves.

Dataflow per core: x (all C routed + S shared tokens, transposed) and the
swiglu gate buffer g live in SBUF for the whole kernel. Phase 1 runs h-tile
OUTER / token-chunk INNER so each w1/w3/sw1/sw3 tile streams from HBM exactly
once. Phase 2 (down-projection) runs against SBUF-resident w2/sw2.
"""

import numpy as np
import ml_dtypes

import concourse.bass as bass
import concourse.mybir as mybir
import concourse.tile as tile
from concourse import bacc
from concourse.bass_utils import run_bass_kernel_spmd

P = 128
N_CORES = 8
F32 = mybir.dt.float32
BF16 = mybir.dt.bfloat16
AF = mybir.ActivationFunctionType
BF16_NP = ml_dtypes.bfloat16

# h-tiles of w1/w3 fetched per DMA (bigger transfers, fewer descriptors)
H_BLOCK = 2


def _chunks(n):
    """Split n tokens into moving-operand chunks of <=512 columns."""
    out = []
    c0 = 0
    while c0 < n:
        cw = min(512, n - c0)
        out.append((c0, cw))
        c0 += cw
    return out


def _ttiles(n):
    """Split n rows into output partition tiles of <=128."""
    out = []
    t0 = 0
    while t0 < n:
        tw = min(P, n - t0)
        out.append((t0, tw))
        t0 += tw
    return out


def build_moe_program(D, H, C, S, use_silu=True):
    """SPMD program: routed expert over C capacity rows + shared expert over
    S token-slice rows. Same NEFF on all 8 cores, per-core input data."""
    nc = bacc.Bacc(
        "TRN2", target_bir_lowering=False, debug=False, num_devices=N_CORES
    )

    KD = D // P
    KH = H // P
    ND = D // 512
    CT = C + S
    CP = (C + P - 1) // P * P  # gate tensor rows (128-multiple)
    hbsz = KD * H_BLOCK * P  # packed cols per h-block

    def din(name, shape, dt=BF16):
        return nc.dram_tensor(name, shape, dt, kind="ExternalInput").ap()

    def dout(name, shape):
        return nc.dram_tensor(name, shape, F32, kind="ExternalOutput").ap()

    xT = din("xT", [P, KD * CT])  # routed tokens then shared tokens
    ge = din("ge", [CP, 1], F32)
    w1 = din("w1", [P, KD * H])
    w3 = din("w3", [P, KD * H])
    w2 = din("w2", [P, KH * D])
    sw1 = din("sw1", [P, KD * H])
    sw3 = din("sw3", [P, KD * H])
    sw2 = din("sw2", [P, KH * D])
    ye = dout("ye", [C, D])
    se = dout("se", [S, D])

    # chunk list: (x-column offset, width, routed?)
    chunk_list = [(c0, cw, True) for c0, cw in _chunks(C)] + [
        (C + c0, cw, False) for c0, cw in _chunks(S)
    ]

    def _wsrc(ap, hb):
        return ap[:, hb * hbsz : (hb + 1) * hbsz].rearrange(
            "p (k m) -> p k m", k=KD
        )

    with tile.TileContext(nc) as tc:
        from contextlib import ExitStack

        with ExitStack() as ctx:
            xpool = ctx.enter_context(tc.tile_pool(name="xT", bufs=1))
            gpool = ctx.enter_context(tc.tile_pool(name="gbuf", bufs=1))
            w2pool = ctx.enter_context(tc.tile_pool(name="w2res", bufs=1))
            wpool = ctx.enter_context(tc.tile_pool(name="wstream", bufs=2))
            spool = ctx.enter_context(tc.tile_pool(name="stemp", bufs=2))
            opool = ctx.enter_context(tc.tile_pool(name="otile", bufs=3))
            gepool = ctx.enter_context(tc.tile_pool(name="gate", bufs=1))
            pp1 = ctx.enter_context(tc.tile_pool(name="ps1", bufs=2, space="PSUM"))
            pp3 = ctx.enter_context(tc.tile_pool(name="ps3", bufs=2, space="PSUM"))
            ppo = ctx.enter_context(tc.tile_pool(name="pso", bufs=2, space="PSUM"))

            # resident activations: [P, KD, CT], loaded per chunk so the
            # first matmuls only wait on the first chunk's columns
            xt = xpool.tile([P, KD, CT], BF16, tag="xt", name="xt")
            xsrc = xT.rearrange("p (k c) -> p k c", k=KD)
            for c0, cw, _ in chunk_list:
                nc.sync.dma_start(xt[:, :, c0 : c0 + cw], xsrc[:, :, c0 : c0 + cw])

            # resident swiglu-gate buffer: [P, KH, CT] bf16
            gt = gpool.tile([P, KH, CT], BF16, tag="gt", name="gt")

            # resident down-projection weights (used only in phase 2; DMAs
            # are issued after the first phase-1 weight tiles below)
            w2res = w2pool.tile([P, KH, D], BF16, tag="w2res", name="w2t")
            sw2res = w2pool.tile([P, KH, D], BF16, tag="sw2res", name="sw2t")

            # per-token gates, [P, CP//P] column-major
            get = gepool.tile([P, CP // P], F32, tag="ge", name="get")
            nc.sync.dma_start(get[:], ge.rearrange("(c p) one -> p (c one)", p=P))

            # ---- phase 1: gt[h, t] = silu(h1T) * h3T, h-block outer ----
            for hb in range(KH // H_BLOCK):
                w1t = wpool.tile([P, KD, H_BLOCK * P], BF16, tag="w1t", name="w1t")
                nc.sync.dma_start(w1t[:], _wsrc(w1, hb))
                w3t = wpool.tile([P, KD, H_BLOCK * P], BF16, tag="w3t", name="w3t")
                nc.sync.dma_start(w3t[:], _wsrc(w3, hb))
                sw1t = wpool.tile([P, KD, H_BLOCK * P], BF16, tag="sw1t", name="sw1t")
                nc.sync.dma_start(sw1t[:], _wsrc(sw1, hb))
                sw3t = wpool.tile([P, KD, H_BLOCK * P], BF16, tag="sw3t", name="sw3t")
                nc.sync.dma_start(sw3t[:], _wsrc(sw3, hb))

                if hb == 0:
                    # queue the phase-2 weights behind the first stream tiles
                    for dn in range(ND):
                        nc.sync.dma_start(
                            w2res[:, :, dn * 512 : (dn + 1) * 512],
                            w2[:, dn * KH * 512 : (dn + 1) * KH * 512].rearrange(
                                "p (k m) -> p k m", k=KH
                            ),
                        )
                        nc.sync.dma_start(
                            sw2res[:, :, dn * 512 : (dn + 1) * 512],
                            sw2[:, dn * KH * 512 : (dn + 1) * KH * 512].rearrange(
                                "p (k m) -> p k m", k=KH
                            ),
                        )

                for hi in range(H_BLOCK):
                    h = hb * H_BLOCK + hi
                    for c0, cw, routed in chunk_list:
                        wa = w1t if routed else sw1t
                        wb = w3t if routed else sw3t
                        p1 = pp1.tile([P, 512], F32, tag="p1", name="p1")[:, :cw]
                        for k in range(KD):
                            nc.tensor.matmul(
                                p1,
                                wa[:, k, hi * P : (hi + 1) * P],
                                xt[:, k, c0 : c0 + cw],
                                start=(k == 0),
                                stop=(k == KD - 1),
                            )
                        p3 = pp3.tile([P, 512], F32, tag="p3", name="p3")[:, :cw]
                        for k in range(KD):
                            nc.tensor.matmul(
                                p3,
                                wb[:, k, hi * P : (hi + 1) * P],
                                xt[:, k, c0 : c0 + cw],
                                start=(k == 0),
                                stop=(k == KD - 1),
                            )
                        gs = gt[:, h, c0 : c0 + cw]
                        if use_silu:
                            s1 = spool.tile([P, 512], F32, tag="s1", name="s1")[:, :cw]
                            nc.scalar.activation(s1, p1, AF.Silu)
                            nc.vector.tensor_mul(gs, s1, p3)
                        else:  # silu(a) = a * sigmoid(a); CoreSim has no Silu
                            s1 = spool.tile([P, 512], F32, tag="s1", name="s1")[:, :cw]
                            nc.scalar.activation(s1, p1, AF.Sigmoid)
                            nc.vector.tensor_mul(s1, s1, p1)
                            nc.vector.tensor_mul(gs, s1, p3)

            # ---- phase 2: ye/se = gT.T @ w2, row-scaled by gate ----
            for sec_routed in (True, False):
                n_rows = C if sec_routed else S
                base = 0 if sec_routed else C
                wres = w2res if sec_routed else sw2res
                out_ap = ye if sec_routed else se
                for t0, tw in _ttiles(n_rows):
                    for dn in range(ND):
                        po = ppo.tile([P, 512], F32, tag="po", name="po")[:tw, :]
                        for kh in range(KH):
                            nc.tensor.matmul(
                                po,
                                gt[:, kh, base + t0 : base + t0 + tw],
                                wres[:, kh, dn * 512 : (dn + 1) * 512],
                                start=(kh == 0),
                                stop=(kh == KH - 1),
                            )
                        ot = opool.tile([P, 512], F32, tag="ot", name="ot")[:tw, :]
                        if sec_routed:
                            nc.vector.tensor_scalar_mul(
                                ot, po, get[:tw, t0 // P : t0 // P + 1]
                            )
                        else:
                            nc.vector.tensor_copy(ot, po)
                        nc.sync.dma_start(
                            out_ap[t0 : t0 + tw, dn * 512 : (dn + 1) * 512], ot
                        )

    nc.compile()
    return nc


_PROGRAM_CACHE = {}
LAST_RESULTS = None  # BassKernelResults of the most recent device run (for test.py)


def _get_program(D, H, C, S):
    key = (D, H, C, S)
    if key not in _PROGRAM_CACHE:
        _PROGRAM_CACHE[key] = build_moe_program(D, H, C, S)
    return _PROGRAM_CACHE[key]


def _pack_xT(xmat):
    """[n, D] row-major tokens -> [P, (D//P)*n] partition-major bf16."""
    n, D = xmat.shape
    KD = D // P
    return np.ascontiguousarray(
        xmat.reshape(n, KD, P).transpose(2, 1, 0).reshape(P, KD * n)
    ).astype(BF16_NP)


def _pack_w13(w):
    """[D, H] -> [P, (D//P)*H] h-block-major bf16: each h-block's weights are
    one contiguous run per partition."""
    Dw, Hw = w.shape
    KD = Dw // P
    nhb = Hw // (H_BLOCK * P)
    return np.ascontiguousarray(
        w.reshape(KD, P, nhb, H_BLOCK * P)
        .transpose(1, 2, 0, 3)
        .reshape(P, KD * Hw)
    ).astype(BF16_NP)


def _pack_w2(w):
    """[H, D] -> [P, H*D//P] dn-major bf16: each 512-wide D-column half is one
    contiguous run per partition."""
    Hw, Dw = w.shape
    KH = Hw // P
    ND = Dw // 512
    return np.ascontiguousarray(
        w.reshape(KH, P, ND, 512).transpose(1, 2, 0, 3).reshape(P, Hw * Dw // P)
    ).astype(BF16_NP)


def _route(xf, w_router):
    """Top-2 routing identical (up to fp rounding) to the jax reference."""
    logits = xf @ w_router.astype(np.float32)  # [T, E]
    # softmax is monotone: top-2 of probs == top-2 of logits, stable ties
    top2 = np.argsort(-logits, axis=1, kind="stable")[:, :2]  # [T, 2]
    lv = np.take_along_axis(logits, top2, axis=1)
    ev = np.exp(lv - lv[:, 0:1])
    gates = ev / ev.sum(axis=1, keepdims=True)  # [T, 2] renormalized
    return top2, gates


def kernel(x, w_router, w1, w3, w2, sw1, sw3, sw2):
    B, SEQ, D = x.shape
    T = B * SEQ
    E, _, H = w1.shape
    assert E == N_CORES
    S = T // N_CORES

    x = np.asarray(x, dtype=np.float32)
    xf = np.ascontiguousarray(x.reshape(T, D))
    top2, gates = _route(xf, np.asarray(w_router, np.float32))

    # per-expert token lists + gate values
    flat_e = top2.ravel()  # slot 2t, 2t+1 -> token t
    flat_g = gates.ravel().astype(np.float32)
    order = np.argsort(flat_e, kind="stable")
    sorted_e = flat_e[order]
    starts = np.searchsorted(sorted_e, np.arange(E + 1))
    tok_by_e = [order[starts[e] : starts[e + 1]] >> 1 for e in range(E)]
    gate_by_e = [flat_g[order[starts[e] : starts[e + 1]]] for e in range(E)]
    counts = np.diff(starts)

    C = max(512, int(counts.max()))
    CP = (C + P - 1) // P * P

    nc = _get_program(D, H, C, S)

    w1 = np.asarray(w1, np.float32)
    w3 = np.asarray(w3, np.float32)
    w2 = np.asarray(w2, np.float32)
    sw1p = _pack_w13(np.asarray(sw1, np.float32))
    sw3p = _pack_w13(np.asarray(sw3, np.float32))
    sw2p = _pack_w2(np.asarray(sw2, np.float32))

    in_maps = []
    for e in range(E):
        n_e = int(counts[e])
        xe_pad = np.zeros((C + S, D), np.float32)
        xe_pad[:n_e] = xf[tok_by_e[e]]
        xe_pad[C:] = xf[e * S : (e + 1) * S]
        ge = np.zeros((CP, 1), np.float32)
        ge[:n_e, 0] = gate_by_e[e]
        in_maps.append(
            {
                "xT": _pack_xT(xe_pad),
                "ge": ge,
                "w1": _pack_w13(w1[e]),
                "w3": _pack_w13(w3[e]),
                "w2": _pack_w2(w2[e]),
                "sw1": sw1p,
                "sw3": sw3p,
                "sw2": sw2p,
            }
        )

    global LAST_RESULTS
    LAST_RESULTS = run_bass_kernel_spmd(nc, in_maps, core_ids=list(range(N_CORES)))
    res = LAST_RESULTS.results

    out = np.empty((T, D), np.float32)
    for c in range(N_CORES):
        out[c * S : (c + 1) * S] = res[c]["se"]
    for e in range(E):
        n_e = int(counts[e])
        if n_e:
            out[tok_by_e[e]] += res[e]["ye"][:n_e]
    return out.reshape(B, SEQ, D)
